# revision 1
# baseline (speedup 1.0000x reference)
"""Distributed Trainium2 kernel for quantized-mixed int8 matmul dequant.

Reference computation (M = K = N = 4096):
    xf = (x - X_ZP) * X_SCALE      # x int32 values in [-128, 127]
    yf = (y - Y_ZP) * Y_SCALE      # y int32 values in [0, 255]
    out = xf @ yf                  # float32 [M, N]

Strategy: 2D-shard the GEMM over 8 NeuronCores as a 4x2 grid
(M split 4 ways, N split 2 ways -> per-core C tile of 1024 x 2048).
Per core, the dequant is fused on-chip: int32 shards are DMA'd in,
shifted by the zero point and cast to bf16 (exact: all shifted values
are integers < 256, exactly representable in bf16), then accumulated
over K in fp32 PSUM via the TensorEngine; the combined scale
X_SCALE*Y_SCALE is applied in the PSUM->SBUF epilogue copy.

x is fed pre-transposed ([K, Mc] int32) so its k-chunks are the
matmul's stationary operand without any on-device transpose.
"""

import numpy as np

import concourse.bacc as bacc
import concourse.mybir as mybir
import concourse.tile as tile
from concourse.bass_utils import run_bass_kernel_spmd

M = K = N = 4096
X_SCALE, X_ZP = 0.03, -66
Y_SCALE, Y_ZP = 0.025, 160
OUT_SCALE = X_SCALE * Y_SCALE

NCORES = 8
MSPLIT, NSPLIT = 4, 2
MC = M // MSPLIT          # 1024 rows of C per core
NCOLS = N // NSPLIT       # 2048 cols of C per core
P = 128                   # partitions / k-chunk size
KC = K // P               # 32 k-chunks
MT = MC // P              # 8 m-tiles (one PSUM bank each)
NF = 512                  # matmul free dim (one PSUM bank at fp32)
NG = NCOLS // NF          # 4 n-groups

_CACHE = {}


def _build():
    nc = bacc.Bacc("TRN2", target_bir_lowering=False, debug=False)
    xt = nc.dram_tensor("xt", [K, MC], mybir.dt.int32, kind="ExternalInput")
    y = nc.dram_tensor("y", [K, NCOLS], mybir.dt.int32, kind="ExternalInput")
    out = nc.dram_tensor("out", [MC, NCOLS], mybir.dt.float32, kind="ExternalOutput")

    with tile.TileContext(nc) as tc:
        with (
            tc.tile_pool(name="xs_pool", bufs=3) as xs_pool,
            tc.tile_pool(name="xb_pool", bufs=KC) as xb_pool,
            tc.tile_pool(name="ys_pool", bufs=6) as ys_pool,
            tc.tile_pool(name="yb_pool", bufs=10) as yb_pool,
            tc.tile_pool(name="ot_pool", bufs=6) as ot_pool,
            tc.tile_pool(name="ps_pool", bufs=8, space="PSUM") as ps_pool,
        ):
            xbf = [None] * KC
            for g in range(NG):
                psums = [None] * MT
                for k in range(KC):
                    if g == 0:
                        # Stream x in once; converted bf16 chunks stay
                        # resident in SBUF for all n-groups.
                        xs = xs_pool.tile([P, MC], mybir.dt.int32, tag="xs",
                                          name=f"xs{k}")
                        nc.sync.dma_start(out=xs[:], in_=xt[k * P:(k + 1) * P, :])
                        xb = xb_pool.tile([P, MC], mybir.dt.bfloat16, tag="xb",
                                          name=f"xb{k}")
                        nc.vector.tensor_scalar_add(out=xb[:], in0=xs[:],
                                                    scalar1=float(-X_ZP))
                        xbf[k] = xb
                    ys = ys_pool.tile([P, NF], mybir.dt.int32, tag="ys",
                                      name=f"ys{g}_{k}")
                    nc.sync.dma_start(out=ys[:],
                                      in_=y[k * P:(k + 1) * P, g * NF:(g + 1) * NF])
                    yb = yb_pool.tile([P, NF], mybir.dt.bfloat16, tag="yb",
                                      name=f"yb{g}_{k}")
                    nc.vector.tensor_scalar_add(out=yb[:], in0=ys[:],
                                                scalar1=float(-Y_ZP))
                    for m in range(MT):
                        if k == 0:
                            psums[m] = ps_pool.tile([P, NF], mybir.dt.float32,
                                                    tag="ps", name=f"ps{g}_{m}")
                        nc.tensor.matmul(psums[m][:],
                                         xbf[k][:, m * P:(m + 1) * P],
                                         yb[:],
                                         start=(k == 0), stop=(k == KC - 1))
                for m in range(MT):
                    ot = ot_pool.tile([P, NF], mybir.dt.float32, tag="ot",
                                      name=f"ot{g}_{m}")
                    # Scale fused into the PSUM->SBUF copy; alternate
                    # engines so bank release isn't serialized on one.
                    if m % 2 == 0:
                        nc.scalar.mul(ot[:], psums[m][:], OUT_SCALE)
                    else:
                        nc.vector.tensor_scalar_mul(out=ot[:], in0=psums[m][:],
                                                    scalar1=OUT_SCALE)
                    nc.sync.dma_start(
                        out=out[m * P:(m + 1) * P, g * NF:(g + 1) * NF],
                        in_=ot[:])
    nc.compile()
    return nc


def _get_nc():
    if "nc" not in _CACHE:
        _CACHE["nc"] = _build()
    return _CACHE["nc"]


def _shard(x, y):
    x = np.ascontiguousarray(np.asarray(x, dtype=np.int32))
    y = np.ascontiguousarray(np.asarray(y, dtype=np.int32))
    xts = [np.ascontiguousarray(x[mi * MC:(mi + 1) * MC, :].T)
           for mi in range(MSPLIT)]
    ys = [np.ascontiguousarray(y[:, ni * NCOLS:(ni + 1) * NCOLS])
          for ni in range(NSPLIT)]
    in_maps = []
    for c in range(NCORES):
        mi, ni = divmod(c, NSPLIT)
        in_maps.append({"xt": xts[mi], "y": ys[ni]})
    return in_maps


def _gather(results):
    out = np.empty((M, N), dtype=np.float32)
    for c in range(NCORES):
        mi, ni = divmod(c, NSPLIT)
        out[mi * MC:(mi + 1) * MC, ni * NCOLS:(ni + 1) * NCOLS] = \
            results[c]["out"]
    return out


def run(x, y, **spmd_kwargs):
    """Run and return (full_output, BassKernelResults)."""
    nc = _get_nc()
    in_maps = _shard(x, y)
    res = run_bass_kernel_spmd(nc, in_maps, core_ids=list(range(NCORES)),
                               **spmd_kwargs)
    return _gather(res.results), res


def kernel(x, y):
    out, _ = run(x, y)
    return out


# revision 5
# speedup vs baseline: 1.0184x; 1.0184x over previous
"""Distributed Trainium2 kernel for quantized-mixed int8 matmul dequant.

Reference computation (M = K = N = 4096):
    xf = (x - X_ZP) * X_SCALE      # x int32 values in [-128, 127]
    yf = (y - Y_ZP) * Y_SCALE      # y int32 values in [0, 255]
    out = xf @ yf                  # float32 [M, N]

Strategy: 2D-shard the GEMM over 8 NeuronCores as a 4x2 grid
(M split 4 ways, N split 2 ways -> per-core C tile of 1024 x 2048).
Per core, the dequant is fused on-chip: int32 shards are DMA'd in,
shifted by the zero point and cast to bf16 (exact: all shifted values
are integers < 256, exactly representable in bf16), then accumulated
over K in fp32 PSUM via the TensorEngine; the combined scale
X_SCALE*Y_SCALE is applied in the PSUM->SBUF epilogue copy.

x is fed pre-transposed ([K, Mc] int32) so its k-chunks are the
matmul's stationary operand without any on-device transpose.
"""

import numpy as np

import concourse.bacc as bacc
import concourse.mybir as mybir
import concourse.tile as tile
from concourse.bass_utils import run_bass_kernel_spmd

M = K = N = 4096
X_SCALE, X_ZP = 0.03, -66
Y_SCALE, Y_ZP = 0.025, 160
OUT_SCALE = X_SCALE * Y_SCALE

NCORES = 8
MSPLIT, NSPLIT = 4, 2
MC = M // MSPLIT          # 1024 rows of C per core
NCOLS = N // NSPLIT       # 2048 cols of C per core
P = 128                   # partitions / k-chunk size
KC = K // P               # 32 k-chunks
MT = MC // P              # 8 m-tiles (one PSUM bank each)
NF = 512                  # matmul free dim (one PSUM bank at fp32)
NG = NCOLS // NF          # 4 n-groups

_CACHE = {}


def _build():
    nc = bacc.Bacc("TRN2", target_bir_lowering=False, debug=False)
    xt = nc.dram_tensor("xt", [K, MC], mybir.dt.int32, kind="ExternalInput")
    y = nc.dram_tensor("y", [K, NCOLS], mybir.dt.int32, kind="ExternalInput")
    out = nc.dram_tensor("out", [MC, NCOLS], mybir.dt.float32, kind="ExternalOutput")

    with tile.TileContext(nc) as tc:
        with (
            tc.tile_pool(name="warm_pool", bufs=1) as warm_pool,
            tc.tile_pool(name="xs_pool", bufs=3) as xs_pool,
            tc.tile_pool(name="xb_pool", bufs=KC) as xb_pool,
            tc.tile_pool(name="ys_pool", bufs=8) as ys_pool,
            tc.tile_pool(name="yb_pool", bufs=12) as yb_pool,
            tc.tile_pool(name="ot_pool", bufs=6) as ot_pool,
            tc.tile_pool(name="ps_pool", bufs=8, space="PSUM") as ps_pool,
        ):
            # PE warm-up: the first ~10us of the kernel are DMA/convert
            # latency with no matmul work, which leaves the PE clock
            # throttled (HAM cold, 1.2 GHz). Burn dummy matmuls on a
            # zeroed tile during that window so the HAM un-throttles
            # and the first real matmuls issue at 2.4 GHz.
            wt = warm_pool.tile([P, NF], mybir.dt.bfloat16, tag="wt")
            nc.vector.memset(wt[:], 0.0)
            wps = ps_pool.tile([64, NF], mybir.dt.float32, tag="ps", name="wps")
            for _ in range(24):
                nc.tensor.matmul(wps[:], wt[:, :64], wt[:], start=True, stop=True)
            xbf = [None] * KC
            for g in range(NG):
                psums = [None] * MT
                for k in range(KC):
                    if g == 0:
                        # Stream x in once; converted bf16 chunks stay
                        # resident in SBUF for all n-groups.
                        xs = xs_pool.tile([P, MC], mybir.dt.int32, tag="xs",
                                          name=f"xs{k}")
                        nc.sync.dma_start(out=xs[:], in_=xt[k * P:(k + 1) * P, :])
                        xb = xb_pool.tile([P, MC], mybir.dt.bfloat16, tag="xb",
                                          name=f"xb{k}")
                        nc.vector.tensor_scalar_add(out=xb[:], in0=xs[:],
                                                    scalar1=float(-X_ZP))
                        xbf[k] = xb
                    ys = ys_pool.tile([P, NF], mybir.dt.int32, tag="ys",
                                      name=f"ys{g}_{k}")
                    nc.sync.dma_start(out=ys[:],
                                      in_=y[k * P:(k + 1) * P, g * NF:(g + 1) * NF])
                    yb = yb_pool.tile([P, NF], mybir.dt.bfloat16, tag="yb",
                                      name=f"yb{g}_{k}")
                    nc.vector.tensor_scalar_add(out=yb[:], in0=ys[:],
                                                scalar1=float(-Y_ZP))
                    for m in range(MT):
                        if k == 0:
                            psums[m] = ps_pool.tile([P, NF], mybir.dt.float32,
                                                    tag="ps", name=f"ps{g}_{m}")
                        nc.tensor.matmul(psums[m][:],
                                         xbf[k][:, m * P:(m + 1) * P],
                                         yb[:],
                                         start=(k == 0), stop=(k == KC - 1))
                for m in range(MT):
                    ot = ot_pool.tile([P, NF], mybir.dt.float32, tag="ot",
                                      name=f"ot{g}_{m}")
                    # Scale fused into the PSUM->SBUF copy; alternate
                    # engines so bank release isn't serialized on one.
                    if m % 2 == 0:
                        nc.scalar.mul(ot[:], psums[m][:], OUT_SCALE)
                    else:
                        nc.vector.tensor_scalar_mul(out=ot[:], in0=psums[m][:],
                                                    scalar1=OUT_SCALE)
                    # Output DMA on the gpsimd queue: on the sync queue its
                    # embedded wait (for the epilogue copy) head-of-line
                    # blocks the next group's y DMA triggers.
                    nc.gpsimd.dma_start(
                        out=out[m * P:(m + 1) * P, g * NF:(g + 1) * NF],
                        in_=ot[:])
    nc.compile()
    return nc


def _get_nc():
    if "nc" not in _CACHE:
        _CACHE["nc"] = _build()
    return _CACHE["nc"]


def _shard(x, y):
    x = np.ascontiguousarray(np.asarray(x, dtype=np.int32))
    y = np.ascontiguousarray(np.asarray(y, dtype=np.int32))
    xts = [np.ascontiguousarray(x[mi * MC:(mi + 1) * MC, :].T)
           for mi in range(MSPLIT)]
    ys = [np.ascontiguousarray(y[:, ni * NCOLS:(ni + 1) * NCOLS])
          for ni in range(NSPLIT)]
    in_maps = []
    for c in range(NCORES):
        mi, ni = divmod(c, NSPLIT)
        in_maps.append({"xt": xts[mi], "y": ys[ni]})
    return in_maps


def _gather(results):
    out = np.empty((M, N), dtype=np.float32)
    for c in range(NCORES):
        mi, ni = divmod(c, NSPLIT)
        out[mi * MC:(mi + 1) * MC, ni * NCOLS:(ni + 1) * NCOLS] = \
            results[c]["out"]
    return out


def run(x, y, **spmd_kwargs):
    """Run and return (full_output, BassKernelResults)."""
    nc = _get_nc()
    in_maps = _shard(x, y)
    res = run_bass_kernel_spmd(nc, in_maps, core_ids=list(range(NCORES)),
                               **spmd_kwargs)
    return _gather(res.results), res


def kernel(x, y):
    out, _ = run(x, y)
    return out
